# revision 30
# baseline (speedup 1.0000x reference)
"""Trainium2 Bass kernel for MABClean (cross-attention block with SetNorm).

Sharding: 8 cores = (batch b in 0..3) x (query-half in 0..1). Each core:
  - gets X[b] (rows permuted so its query half comes first) and Y[b], both
    transposed to feature-major [256, 2048] bf16 layout,
  - computes SetNorm stats of X/Y on-device (bn_stats on bf16, quake-rsqrt
    on DVE so the ACT table set never leaves exp), Q for its 1024 queries,
    full K/V,
  - attention: per (qt, hg) phase, scores via 2-way row-tiled K=32 matmuls
    per head-pair (tile_position), exp SPLIT across engines: ACT does
    native exp -> fp8e4 for heads {0,1} of the group, DVE does a
    Schraudolph bit-trick exp -> uint8/fp8e5 for heads {2,3}; AV as fp8
    DoubleRow matmuls (ones-column producing softmax denominators),
  - O/residual, AllReduces the final SetNorm (sum, sumsq) with its pair
    core, applies norm+relu+res projection, returns H^T half.
"""

import math

import numpy as np

import concourse.bass as bass
import concourse.tile as tile
from concourse import bacc, mybir
from concourse.bass_utils import run_bass_kernel_spmd

F32 = mybir.dt.float32
BF16 = mybir.dt.bfloat16
FP8 = mybir.dt.float8e4
FP8E5 = mybir.dt.float8e5
U8 = mybir.dt.uint8
U32 = mybir.dt.uint32
AF = mybir.ActivationFunctionType
ALU = mybir.AluOpType
PM = mybir.MatmulPerfMode

P = 128
D = 256      # feature dim (dX = dY)
NQ = 1024    # queries per core
NK = 2048    # keys
H = 8        # heads
DH = 32      # head dim
NKC = NK // P    # 16 key chunks
NT = NKC // 2    # 8 kc-pairs for DoubleRow AV
HB = 48          # head block in VO: 32 V dims + 1 ones + 15 pad
EPS = 1e-5
LOG2E = 1.4426950408889634
# exp(x) ~ fp8e5 bits = x*scale*4*log2e + 60 - 0.2  (RNE int cast, measured)
SCHRA_A = 0.0625 * 4.0 * LOG2E
SCHRA_B = 59.8
QMAGIC = 0x5F3759DF
WNAMES = ["WvT", "WkT", "WqT", "WoT", "WresT"]
PNAMES = ["bq", "bk", "bo", "bres", "nqw", "nqb", "nkw", "nkb",
          "n0w", "n0b", "rsq", "cq", "rsk", "ck"]

_CACHE = {}


def build_module():
    nc = bacc.Bacc("TRN2", target_bir_lowering=False, debug=False,
                   num_devices=8)

    # ---- DRAM I/O ----
    XT = nc.dram_tensor("XT", [D, NK], BF16, kind="ExternalInput").ap()
    YT = nc.dram_tensor("YT", [D, NK], BF16, kind="ExternalInput").ap()
    WCAT = nc.dram_tensor("WCAT", [D, 5 * D], BF16, kind="ExternalInput").ap()
    PVEC = nc.dram_tensor("PVEC", [D, len(PNAMES)], F32,
                          kind="ExternalInput").ap()
    BVR = nc.dram_tensor("BVR", [1, D], BF16, kind="ExternalInput").ap()
    OUT = nc.dram_tensor("OUT", [D, NQ], F32, kind="ExternalOutput").ap()

    with tile.TileContext(nc) as tc:
        with (
            tc.tile_pool(name="persist", bufs=1) as pe,
            tc.tile_pool(name="etpool", bufs=4) as etp,
            tc.tile_pool(name="small", bufs=4) as sm,
            tc.tile_pool(name="stA", bufs=1, space="PSUM") as stA,
            tc.tile_pool(name="stB", bufs=1, space="PSUM") as stB,
            tc.tile_pool(name="opool", bufs=1, space="PSUM") as op,
            tc.tile_pool(name="dram", bufs=2, space="DRAM") as dp,
        ):
            # ---- constants ----
            ones_col = pe.tile([P, 1], F32, tag="ones_col", name="ones_col")
            nc.gpsimd.memset(ones_col[:], 1.0)
            ones_row = pe.tile([1, P], F32, tag="ones_row", name="ones_row")
            nc.gpsimd.memset(ones_row[:], 1.0)
            zero_col = pe.tile([P, 1], F32, tag="zero_col", name="zero_col")
            nc.gpsimd.memset(zero_col[:], 0.0)
            ones1 = pe.tile([1, P], BF16, tag="ones1", name="ones1")
            nc.gpsimd.memset(ones1[:], 1.0)
            eps_t = sm.tile([1, 1], F32, tag="eps_t", name="eps_t")
            nc.vector.memset(eps_t[:], EPS)
            magic_t = sm.tile([1, 1], U32, tag="magic_t", name="magic_t")
            nc.vector.memset(magic_t[:], QMAGIC)
            mh_t = sm.tile([1, 1], F32, tag="mh_t", name="mh_t")
            nc.vector.memset(mh_t[:], -1.5)

            # preload the exp table set before any other ACT work
            dummy_e = sm.tile([1, 1], F32, tag="dummy_e", name="dummy_e")
            nc.scalar.activation(out=dummy_e[:], in_=eps_t[:], func=AF.Exp,
                                 bias=0.0, scale=1.0)

            # VO: [P, kc-pair, kc-parity, 8 heads x 48]; 32 V dims + ones
            # col (softmax denominator) + 15 zero pad per head block.
            VO = pe.tile([P, NT, 2, H * HB], FP8, tag="VO", name="VO")
            vview = VO[:].rearrange("p t s (h e) -> p t s h e", e=HB)
            nc.gpsimd.memset(vview[:, :, :, :, 32:HB], 0.0)
            nc.gpsimd.memset(vview[:, :, :, :, 32:33], 1.0)

            # ---- load inputs (Y first: longest dependency chain) ----
            YTs = [pe.tile([P, NK], BF16, tag=f"YT{i}", name=f"YT{i}")
                   for i in range(2)]
            XTs = [pe.tile([P, NK], BF16, tag=f"XT{i}", name=f"XT{i}")
                   for i in range(2)]
            qs = [nc.sync, nc.scalar]
            qi = 0
            for h_ in range(2):
                for i in range(2):
                    qs[qi % 2].dma_start(
                        out=YTs[i][:, h_ * 1024:(h_ + 1) * 1024],
                        in_=YT[i * P:(i + 1) * P, h_ * 1024:(h_ + 1) * 1024])
                    qi += 1
            wcat = [pe.tile([P, 5 * D], BF16, tag=f"wcat{i}", name=f"wcat{i}")
                    for i in range(2)]
            for i in range(2):
                qs[qi % 2].dma_start(out=wcat[i][:],
                                     in_=WCAT[i * P:(i + 1) * P, :])
                qi += 1
            Ws = {n: [wcat[i][:, k * D:(k + 1) * D] for i in range(2)]
                  for k, n in enumerate(WNAMES)}
            pvt = [pe.tile([P, len(PNAMES)], F32, tag=f"pv{i}", name=f"pv{i}")
                   for i in range(2)]
            for i in range(2):
                qs[qi % 2].dma_start(out=pvt[i][:],
                                     in_=PVEC[i * P:(i + 1) * P, :])
                qi += 1
            pp = {n: [pvt[i][:, k:k + 1] for i in range(2)]
                  for k, n in enumerate(PNAMES)}
            bvr = pe.tile([1, D], BF16, tag="bvr", name="bvr")
            nc.sync.dma_start(out=bvr[:], in_=BVR)
            for h_ in range(2):
                for i in range(2):
                    qs[qi % 2].dma_start(
                        out=XTs[i][:, h_ * 1024:(h_ + 1) * 1024],
                        in_=XT[i * P:(i + 1) * P, h_ * 1024:(h_ + 1) * 1024])
                    qi += 1

            # ---- PE warm-up: keep the HAM clocked up until real work ----
            warm_sb = pe.tile([P, 512], BF16, tag="warm_sb", name="warm_sb")
            nc.vector.memset(warm_sb[:], 0.0)
            for i in range(10):
                pw = op.tile([P, 512], F32, tag="Os0", name="warm")
                nc.tensor.matmul(pw[:], lhsT=warm_sb[:, 0:P],
                                 rhs=warm_sb[:], start=True, stop=True)

            # ---- pre-warm the collective ring with a dummy AllReduce ----
            cc_sb = sm.tile([1, P], F32, tag="cc_sb", name="cc_sb")
            nc.vector.memset(cc_sb[:], 0.0)
            warm_in = dp.tile([1, P], F32)
            warm_out = dp.tile([1, P], F32)
            nc.gpsimd.dma_start(out=warm_in[:], in_=cc_sb[:])
            nc.gpsimd.collective_compute(
                "AllReduce", ALU.add,
                replica_groups=[[0, 1], [2, 3], [4, 5], [6, 7]],
                ins=[warm_in.opt()], outs=[warm_out.opt()])

            # ---- helpers ----
            def quake_rsqrt(dst, var, tagp):
                """dst [1,1] f32 <- 1/sqrt(var), DVE-only (no ACT tables)."""
                chalf = sm.tile([1, 1], F32, tag=f"qh_{tagp}",
                                name=f"qh_{tagp}")
                nc.vector.tensor_scalar_mul(out=chalf[:], in0=var, scalar1=-0.5)
                yb = dst.bitcast(U32)
                t1 = sm.tile([1, 1], U32, tag=f"qt_{tagp}", name=f"qt_{tagp}")
                nc.vector.tensor_scalar(out=t1[:], in0=var.bitcast(U32),
                                        scalar1=1, scalar2=None,
                                        op0=ALU.logical_shift_right)
                nc.vector.tensor_tensor(out=yb, in0=magic_t[:], in1=t1[:],
                                        op=ALU.subtract)
                y2 = sm.tile([1, 1], F32, tag=f"qy_{tagp}", name=f"qy_{tagp}")
                t2 = sm.tile([1, 1], F32, tag=f"qz_{tagp}", name=f"qz_{tagp}")
                for _ in range(1):
                    nc.vector.tensor_mul(out=y2[:], in0=dst, in1=dst)
                    nc.vector.scalar_tensor_tensor(
                        out=t2[:], in0=y2[:], scalar=chalf[:, 0:1],
                        in1=mh_t[:], op0=ALU.mult, op1=ALU.subtract)
                    nc.vector.tensor_mul(out=dst, in0=dst, in1=t2[:])

            def finish_from_st(st, nparts, tagp, psum_tag):
                """st [P,n,6] -> bc [P,2]: col0 -mean, col1 1/sd."""
                mv = sm.tile([P, 2], F32, tag=f"mv_{tagp}", name=f"mv_{tagp}")
                nc.vector.bn_aggr(out=mv[:], in_=st[:])
                ms2 = sm.tile([P, 2], F32, tag=f"ms2_{tagp}",
                              name=f"ms2_{tagp}")
                nc.vector.tensor_copy(out=ms2[:, 0:1], in_=mv[:, 0:1])
                nc.vector.scalar_tensor_tensor(
                    out=ms2[:, 1:2], in0=mv[:, 0:1], scalar=mv[:, 0:1],
                    in1=mv[:, 1:2], op0=ALU.mult, op1=ALU.add)
                psum_s = (stA if psum_tag == "STA" else stB).tile(
                    [P, 2, 512], F32, tag=psum_tag, name=f"ps_{tagp}")[:, 0, :]
                nc.tensor.matmul(psum_s[0:1, 0:2], lhsT=ones_col[:],
                                 rhs=ms2[:], start=True, stop=True)
                ssum = sm.tile([1, 2], F32, tag=f"ssum_{tagp}",
                               name=f"ssum_{tagp}")
                nc.vector.tensor_copy(out=ssum[:], in_=psum_s[0:1, 0:2])
                return finish_stats(ssum, nparts, tagp, psum_tag)

            def finish_stats(ssum, nparts, tagp, psum_tag):
                """ssum [1,2] (sum mean, sum ex2) -> bc [P,2]."""
                st2 = sm.tile([1, 2], F32, tag=f"st2_{tagp}", name=f"st2_{tagp}")
                nc.vector.tensor_scalar_mul(out=st2[:], in0=ssum[:],
                                            scalar1=1.0 / nparts)
                negvar = sm.tile([1, 1], F32, tag=f"nv_{tagp}", name=f"nv_{tagp}")
                nc.vector.scalar_tensor_tensor(
                    out=negvar[:], in0=st2[:, 0:1], scalar=st2[:, 0:1],
                    in1=st2[:, 1:2], op0=ALU.mult, op1=ALU.subtract)
                var = sm.tile([1, 1], F32, tag=f"var_{tagp}", name=f"var_{tagp}")
                nc.vector.tensor_scalar(out=var[:], in0=negvar[:],
                                        scalar1=-1.0, scalar2=EPS,
                                        op0=ALU.mult, op1=ALU.add)
                inv = sm.tile([1, 2], F32, tag=f"inv_{tagp}", name=f"inv_{tagp}")
                quake_rsqrt(inv[:, 1:2], var[:], tagp)
                nc.vector.tensor_scalar_mul(out=inv[:, 0:1], in0=st2[:, 0:1],
                                            scalar1=-1.0)
                pb = (stA if psum_tag == "STA" else stB).tile(
                    [P, 2, 512], F32, tag=psum_tag,
                    name=f"pb_{tagp}")[:, 0, 0:2]
                nc.tensor.matmul(pb, lhsT=ones_row[:], rhs=inv[:],
                                 start=True, stop=True)
                bc = sm.tile([P, 2], F32, tag=f"bc_{tagp}", name=f"bc_{tagp}")
                nc.vector.tensor_copy(out=bc[:], in_=pb)
                return bc

            def factors(bc, wname, bname, tagp):
                """Per-chunk scale a = w*inv, shift b = a*(-mean) + beta."""
                outs = []
                for i in range(2):
                    a = pe.tile([P, 1], F32, tag=f"a_{tagp}{i}", name=f"a_{tagp}{i}")
                    nc.vector.tensor_scalar_mul(out=a[:], in0=pp[wname][i],
                                                scalar1=bc[:, 1:2])
                    b = pe.tile([P, 1], F32, tag=f"b_{tagp}{i}", name=f"b_{tagp}{i}")
                    nc.vector.scalar_tensor_tensor(
                        out=b[:], in0=a[:], scalar=bc[:, 0:1],
                        in1=pp[bname][i], op0=ALU.mult, op1=ALU.add)
                    outs.append((a, b))
                return outs

            def fold_factors(bc, rsn, cn, tagp):
                """beta = cn + (-mean*inv)*rsn per chunk; alpha = inv."""
                g = sm.tile([P, 1], F32, tag=f"g_{tagp}", name=f"g_{tagp}")
                nc.vector.tensor_mul(out=g[:], in0=bc[:, 0:1], in1=bc[:, 1:2])
                outs = []
                for i in range(2):
                    b = pe.tile([P, 1], F32, tag=f"fb_{tagp}{i}",
                                name=f"fb_{tagp}{i}")
                    nc.vector.scalar_tensor_tensor(
                        out=b[:], in0=pp[rsn][i], scalar=g[:, 0:1],
                        in1=pp[cn][i], op0=ALU.mult, op1=ALU.add)
                    outs.append(b)
                return outs

            # ---- SetNorm stats (DVE) ----
            st_y4 = sm.tile([P, 8, 6], F32, tag="st_y4", name="st_y4")
            for i in range(2):
                yv = YTs[i][:].rearrange("p (n f) -> p n f", f=512)
                for c in range(4):
                    nc.vector.bn_stats(out=st_y4[:, i * 4 + c, :],
                                       in_=yv[:, c, :])
            bcY = finish_from_st(st_y4, P, "y", "STA")
            bK = fold_factors(bcY, "rsk", "ck", "y")

            # ---- K projection -> KTs bf16 (scale-adds on ACT) ----
            KTs = [pe.tile([P, NK], BF16, tag=f"KT{i}", name=f"KT{i}")
                   for i in range(2)]
            pu = 0
            for fo in range(2):
                for nt in range(4):
                    pk = op.tile([P, 512], F32, tag=f"Os{pu % 4}",
                                 name="pproj")
                    pu += 1
                    for cc in range(2):
                        nc.tensor.matmul(
                            pk[:],
                            lhsT=Ws["WkT"][cc][:, fo * P:(fo + 1) * P],
                            rhs=YTs[cc][:, nt * 512:(nt + 1) * 512],
                            start=(cc == 0), stop=(cc == 1))
                    nc.scalar.activation(
                        out=KTs[fo][:, nt * 512:(nt + 1) * 512],
                        in_=pk[:], func=AF.Identity, bias=bK[fo][:],
                        scale=bcY[:, 1:2])

            st_x4 = sm.tile([P, 8, 6], F32, tag="st_x4", name="st_x4")
            for i in range(2):
                xv = XTs[i][:].rearrange("p (n f) -> p n f", f=512)
                for c in range(4):
                    nc.vector.bn_stats(out=st_x4[:, i * 4 + c, :],
                                       in_=xv[:, c, :])
            bcX = finish_from_st(st_x4, P, "x", "STB")
            bQ = fold_factors(bcX, "rsq", "cq", "x")

            # ---- Q projection -> QTs bf16 (scale-adds on ACT) ----
            QTs = [pe.tile([P, NQ], BF16, tag=f"QT{i}", name=f"QT{i}")
                   for i in range(2)]
            for qt in range(2):
                for fo in range(2):
                    pq = op.tile([P, 512], F32, tag=f"Os{pu % 4}",
                                 name="pproj")
                    pu += 1
                    for cc in range(2):
                        nc.tensor.matmul(
                            pq[:],
                            lhsT=Ws["WqT"][cc][:, fo * P:(fo + 1) * P],
                            rhs=XTs[cc][:, qt * 512:(qt + 1) * 512],
                            start=(cc == 0), stop=(cc == 1))
                    nc.scalar.activation(
                        out=QTs[fo][:, qt * 512:(qt + 1) * 512],
                        in_=pq[:], func=AF.Identity, bias=bQ[fo][:],
                        scale=bcX[:, 1:2])

            # ---- V projection -> VO (bv folded in via broadcast start-MM);
            #      PSUM->fp8 copies on ACT after the K/Q scale-adds
            def emit_v(kc):
                pv = op.tile([P, 512], F32, tag=f"Os{kc % 4}",
                             name="pproj")[:, 0:D]
                nc.tensor.matmul(pv, lhsT=ones1[:, 0:P], rhs=bvr[:],
                                 start=True, stop=False)
                for cc in range(2):
                    nc.tensor.matmul(
                        pv, lhsT=YTs[cc][:, kc * P:(kc + 1) * P],
                        rhs=Ws["WvT"][cc][:],
                        start=False, stop=(cc == 1))
                nc.scalar.activation(
                    out=vview[:, kc // 2, kc % 2, :, 0:32],
                    in_=pv.rearrange("p (h e) -> p h e", e=32),
                    func=AF.Copy, bias=0.0, scale=1.0)
            for kc in range(NKC):
                emit_v(kc)

            # ---- attention ----
            OcatT = [pe.tile([P, NQ], BF16, tag=f"Ocat{i}", name=f"Ocat{i}")
                     for i in range(2)]
            H1T = [pe.tile([P, NQ], F32, tag=f"H1T{i}", name=f"H1T{i}")
                   for i in range(2)]

            def emit_phase(qt, hg):
                """Scores -> exp (ACT pair01 / DVE pair23) -> AV DoubleRow."""
                Os = [op.tile([P, 512], F32, tag=f"Os{g}", name=f"Os{g}")
                      for g in range(4)]
                ET01s, ET23s = [None] * NT, [None] * NT

                def emit_av(t, j):
                    h = 4 * hg + j
                    if j < 2:
                        rhs = ET01s[t][:, j, :, :]
                    else:
                        rhs = ET23s[t][:, j - 2, :, :].bitcast(FP8E5)
                    nc.tensor.matmul(
                        Os[j][0:HB, :],
                        lhsT=VO[:, t, :, HB * h:HB * h + HB],
                        rhs=rhs,
                        start=(t == 0), stop=(t == NT - 1),
                        perf_mode=PM.DoubleRow)

                for t in range(NT):
                    STa = stA.tile([P, 2, 512], F32, tag="STA", name="STA")
                    STb = stB.tile([P, 2, 512], F32, tag="STB", name="STB")
                    ET01s[t] = etp.tile([P, 2, 2, 512], FP8, tag="ET01",
                                        name="ET01")
                    ET23s[t] = etp.tile([P, 2, 2, 512], U8, tag="ET23",
                                        name="ET23")
                    for s in range(2):
                        kc = 2 * t + s
                        if s == 1 and t > 0:
                            emit_av(t - 1, 0)
                            emit_av(t - 1, 1)
                        for j in range(2):
                            nc.tensor.matmul(
                                STa[:, j, :],
                                lhsT=KTs[hg][32 * j:32 * j + 32,
                                             kc * P:(kc + 1) * P],
                                rhs=QTs[hg][32 * j:32 * j + 32,
                                            qt * 512:(qt + 1) * 512],
                                start=True, stop=True,
                                tile_position=(32 * j, 0))
                        if s == 1 and t > 0:
                            emit_av(t - 1, 2)
                            emit_av(t - 1, 3)
                        for j in range(2, 4):
                            nc.tensor.matmul(
                                STb[:, j - 2, :],
                                lhsT=KTs[hg][32 * j:32 * j + 32,
                                             kc * P:(kc + 1) * P],
                                rhs=QTs[hg][32 * j:32 * j + 32,
                                            qt * 512:(qt + 1) * 512],
                                start=True, stop=True,
                                tile_position=(32 * j, 0))
                        nc.scalar.activation(out=ET01s[t][:, :, s, :],
                                             in_=STa[:], func=AF.Exp,
                                             bias=zero_col[:], scale=0.0625)
                        if t < NT - 1:
                            nc.vector.tensor_scalar(out=ET23s[t][:, :, s, :],
                                                    in0=STb[:],
                                                    scalar1=SCHRA_A,
                                                    scalar2=SCHRA_B,
                                                    op0=ALU.mult, op1=ALU.add)
                        else:
                            # rebalance: last t's pair23 exp on ACT (true
                            # exp, value-converted to e5m2)
                            nc.scalar.activation(
                                out=ET23s[t][:, :, s, :].bitcast(FP8E5),
                                in_=STb[:], func=AF.Exp,
                                bias=zero_col[:], scale=0.0625)
                for j in range(4):
                    emit_av(NT - 1, j)

                # denominators -> reciprocal -> broadcast -> fused rescale
                dall = sm.tile([1, 4, 512], F32, tag="dall", name="dall")
                for j in range(2):
                    nc.scalar.activation(out=dall[:, j, :],
                                         in_=Os[j][32:33, :],
                                         func=AF.Copy, bias=0.0, scale=1.0)
                for j in range(2, 4):
                    nc.vector.tensor_copy(out=dall[:, j, :],
                                          in_=Os[j][32:33, :])
                dsb = sm.tile([32, 64], F32, tag="dsb", name="dsb")
                nc.sync.dma_start(
                    out=dsb[:], in_=dall[:].rearrange("p a b -> p (a b)"))
                rsb = sm.tile([32, 64], F32, tag="rsb", name="rsb")
                nc.vector.reciprocal(out=rsb[:], in_=dsb[:])
                rdr = dp.tile([1, 4 * 512], F32, name="rdr")
                nc.sync.dma_start(
                    out=bass.AP(tensor=rdr.tensor, offset=rdr.offset,
                                ap=[[64, 32], [1, 64]]),
                    in_=rsb[:])
                rball = sm.tile([32, 4, 512], F32, tag="rball", name="rball")
                nc.sync.dma_start(
                    out=rball[:],
                    in_=bass.AP(tensor=rdr.tensor, offset=rdr.offset,
                                ap=[[0, 32], [512, 4], [1, 512]]))
                for j in range(4):
                    nc.vector.tensor_tensor(
                        out=OcatT[hg][32 * j:32 * j + 32,
                                      qt * 512:(qt + 1) * 512],
                        in0=Os[j][0:32, :], in1=rball[:, j, :],
                        op=ALU.mult)

            def emit_oproj(qt):
                """O projection + residual for one qt half."""
                for fo in range(2):
                    po = stA.tile([P, 2, 512], F32, tag="STA",
                                  name="po")[:, 0, :]
                    for cc in range(2):
                        nc.tensor.matmul(
                            po[:],
                            lhsT=Ws["WoT"][cc][:, fo * P:(fo + 1) * P],
                            rhs=OcatT[cc][:, qt * 512:(qt + 1) * 512],
                            start=(cc == 0), stop=(cc == 1))
                    nc.vector.scalar_tensor_tensor(
                        out=H1T[fo][:, qt * 512:(qt + 1) * 512], in0=po[:],
                        scalar=pp["bo"][fo],
                        in1=XTs[fo][:, qt * 512:(qt + 1) * 512],
                        op0=ALU.add, op1=ALU.add)

            st_h = sm.tile([P, 4, 6], F32, tag="st_h", name="st_h")

            def mid_work():
                emit_oproj(0)
                for i in range(2):
                    nc.vector.bn_stats(out=st_h[:, i, :],
                                       in_=H1T[i][:, 0:512])

            def cc_reduce(ss, tagp):
                csb = sm.tile([1, P], F32, tag=f"csb_{tagp}",
                              name=f"csb_{tagp}")
                nc.vector.memset(csb[:], 0.0)
                nc.vector.tensor_copy(out=csb[:, 0:2], in_=ss[:])
                cin = dp.tile([1, P], F32)
                cout = dp.tile([1, P], F32)
                nc.sync.dma_start(out=cin[:], in_=csb[:])
                nc.gpsimd.collective_compute(
                    "AllReduce", ALU.add,
                    replica_groups=[[0, 1], [2, 3], [4, 5], [6, 7]],
                    ins=[cin.opt()], outs=[cout.opt()])
                red = sm.tile([1, 2], F32, tag=f"ccr_{tagp}",
                              name=f"ccr_{tagp}")
                nc.sync.dma_start(out=red[:], in_=cout[0:1, 0:2])
                return red

            emit_phase(0, 0)
            emit_phase(0, 1)
            emit_phase(1, 0)
            mid_work()
            emit_phase(1, 1)
            emit_oproj(1)

            # ---- final setnorm (cross-core) + relu + res projection ----
            for i in range(2):
                nc.vector.bn_stats(out=st_h[:, 2 + i, :],
                                   in_=H1T[i][:, 512:1024])
            mv_h = sm.tile([P, 2], F32, tag="mv_h", name="mv_h")
            nc.vector.bn_aggr(out=mv_h[:], in_=st_h[:])
            ms2_h = sm.tile([P, 2], F32, tag="ms2_h", name="ms2_h")
            nc.vector.tensor_copy(out=ms2_h[:, 0:1], in_=mv_h[:, 0:1])
            nc.vector.scalar_tensor_tensor(
                out=ms2_h[:, 1:2], in0=mv_h[:, 0:1], scalar=mv_h[:, 0:1],
                in1=mv_h[:, 1:2], op0=ALU.mult, op1=ALU.add)
            psum_h = stA.tile([P, 2, 512], F32, tag="STA",
                              name="psh")[:, 0, :]
            nc.tensor.matmul(psum_h[0:1, 0:2], lhsT=ones_col[:],
                             rhs=ms2_h[:], start=True, stop=True)
            ssH = sm.tile([1, 2], F32, tag="ssH", name="ssH")
            nc.vector.tensor_copy(out=ssH[:], in_=psum_h[0:1, 0:2])
            red = cc_reduce(ssH, "h")
            bcH = finish_stats(red, 2 * P, "h", "STB")
            fH = factors(bcH, "n0w", "n0b", "h")
            RT = []
            for i in range(2):
                t = pe.tile([P, NQ], BF16, tag=f"RT{i}", name=f"RT{i}")
                nc.scalar.activation(out=t[:], in_=H1T[i][:], func=AF.Relu,
                                     bias=fH[i][1][:], scale=fH[i][0][:])
                RT.append(t)
            OutT = [pe.tile([P, NQ], F32, tag=f"OutT{i}", name=f"OutT{i}")
                    for i in range(2)]
            for qt in range(2):
                for fo in range(2):
                    pr = stA.tile([P, 2, 512], F32, tag="STA",
                                  name="pres")[:, 0, :]
                    for cc in range(2):
                        nc.tensor.matmul(
                            pr[:],
                            lhsT=Ws["WresT"][cc][:, fo * P:(fo + 1) * P],
                            rhs=RT[cc][:, qt * 512:(qt + 1) * 512],
                            start=(cc == 0), stop=(cc == 1))
                    nc.vector.scalar_tensor_tensor(
                        out=OutT[fo][:, qt * 512:(qt + 1) * 512], in0=pr[:],
                        scalar=pp["bres"][fo],
                        in1=H1T[fo][:, qt * 512:(qt + 1) * 512],
                        op0=ALU.add, op1=ALU.add)
                    nc.scalar.dma_start(
                        out=OUT[fo * P:(fo + 1) * P,
                                qt * 512:(qt + 1) * 512],
                        in_=OutT[fo][:, qt * 512:(qt + 1) * 512])

    nc.compile()
    return nc


def _prep_inputs(X, Y, Wq, bq, Wk, bk, Wv, bv, Wo, bo, Wres, bres,
                 nq_w, nq_b, nk_w, nk_b, n0_w, n0_b):
    c = np.ascontiguousarray
    import ml_dtypes
    bf = ml_dtypes.bfloat16
    WkTf = Wk.T * nk_w[:, None]
    WqTf = Wq.T * nq_w[:, None]
    wcat = np.concatenate([Wv.T, WkTf, WqTf, Wo.T, Wres.T],
                          axis=1).astype(bf)
    pv = {"bq": bq, "bk": bk, "bo": bo, "bres": bres, "nqw": nq_w,
          "nqb": nq_b, "nkw": nk_w, "nkb": nk_b, "n0w": n0_w, "n0b": n0_b,
          "rsq": WqTf.sum(axis=0), "cq": Wq @ nq_b + bq,
          "rsk": WkTf.sum(axis=0), "ck": Wk @ nk_b + bk}
    pvec = np.stack([pv[n] for n in PNAMES], axis=1).astype(np.float32)
    shared = {
        "WCAT": c(wcat),
        "PVEC": c(pvec),
        "BVR": c(bv.astype(bf).reshape(1, D)),
    }
    in_maps = []
    for core in range(8):
        b, half = core // 2, core % 2
        Xb = np.asarray(X[b], dtype=np.float32)
        perm = np.concatenate(
            [Xb[half * NQ:(half + 1) * NQ], Xb[(1 - half) * NQ:
                                               (2 - half) * NQ]], axis=0)
        m = dict(shared)
        m["XT"] = c(perm.T.astype(bf))
        m["YT"] = c(np.asarray(Y[b], dtype=np.float32).T.astype(bf))
        in_maps.append(m)
    return in_maps


def run(in_maps, trace=False):
    if "nc" not in _CACHE:
        _CACHE["nc"] = build_module()
    return run_bass_kernel_spmd(_CACHE["nc"], in_maps,
                                core_ids=list(range(8)), trace=trace)


def kernel(**inputs):
    in_maps = _prep_inputs(**inputs)
    res = run(in_maps, trace=False)
    B = 4
    out = np.empty((B, 2 * NQ, D), dtype=np.float32)
    for core in range(8):
        b, half = core // 2, core % 2
        out[b, half * NQ:(half + 1) * NQ, :] = res.results[core]["OUT"].T
    return out
